# revision 1
# baseline (speedup 1.0000x reference)
# Fused attention block (LeViT-style) for Trainium2, 8 NeuronCores, data-parallel over batch.
#
# reference computation (B=16, N=784, DIM=512, H=8, KD=64, VD=256):
#   qkv = BN(x @ qkv_w.T); split q,k,v per head
#   attn = softmax(q @ k.T * KD**-0.5 + attention_biases[:, bias_idxs])
#   out  = BN(silu(attn @ v reshaped) @ proj_w.T)
#
# Strategy:
#  - batch-parallel: 2 batches per core, weights/bias tables replicated, no collectives
#  - BN folded into weights on host; softmax scale folded into q weights
#  - all matmul operands bf16 (PSUM accumulation fp32), softmax pipeline fp32
#  - scores computed transposed (S^T[j,i]); bias table is symmetric so bias adds unchanged
#  - softmax denominator from an extra ones-column in v (col 256 of each head block)
#  - unstabilized softmax (scores empirically bounded ~|10|, exp is safe in fp32)
#  - heads processed in even/odd pairs at PE row bases 0/64: the K=64 score matmuls
#    of the two heads occupy disjoint row groups and execute concurrently
#  - silu applied in bulk per t-chunk so the h-loop ACT stream is pure Exp (no
#    activation-table thrash)

import numpy as np
import ml_dtypes

B, N, DIM = 16, 784, 512
H, KD, VD = 8, 64, 256
RES = 28
EPS = 1e-5
SCALE = KD ** -0.5
NCORES = 8
BL = B // NCORES          # batches per core
VDA = VD + 1              # v head block with ones column
OVW = H * VDA             # 2056
NJP = 896                 # padded j extent (7 * 128)

# t/j chunking over N=784: six 128-chunks + one 16-chunk
CHUNKS = [(i * 128, min(128, N - i * 128)) for i in range((N + 127) // 128)]
ITILES = [(0, 512), (512, N - 512)]   # free-dim tiles for 784 (<=512 per PSUM bank)

_CACHE = {}


def _build_nc():
    from contextlib import ExitStack
    import concourse.bacc as bacc
    import concourse.tile as tile
    from concourse import mybir

    bf = mybir.dt.bfloat16
    f32 = mybir.dt.float32
    AF = mybir.ActivationFunctionType
    MULT = mybir.AluOpType.mult

    nc = bacc.Bacc("TRN2", target_bir_lowering=False, debug=False)

    xT = nc.dram_tensor("xT", [BL, DIM, N], bf, kind="ExternalInput").ap()
    wqk = nc.dram_tensor("wqk", [128, 4, 1024], bf, kind="ExternalInput").ap()
    wv = nc.dram_tensor("wv", [128, 4, H * VD], bf, kind="ExternalInput").ap()
    wp = nc.dram_tensor("wp", [128, 16, DIM], bf, kind="ExternalInput").ap()
    bqk = nc.dram_tensor("bqk", [128, 8], f32, kind="ExternalInput").ap()
    bv = nc.dram_tensor("bv", [1, H * VD], bf, kind="ExternalInput").ap()
    bp = nc.dram_tensor("bp", [1, DIM], bf, kind="ExternalInput").ap()
    biast = nc.dram_tensor("biast", [H, NJP, N], bf, kind="ExternalInput").ap()
    ones = nc.dram_tensor("ones", [1, 128], bf, kind="ExternalInput").ap()
    ident = nc.dram_tensor("ident", [128, 128], bf, kind="ExternalInput").ap()
    out = nc.dram_tensor("out", [BL, N, DIM], f32, kind="ExternalOutput").ap()

    import concourse.bass as bass

    with ExitStack() as ctx:
        tc = ctx.enter_context(tile.TileContext(nc))
        consts = ctx.enter_context(tc.tile_pool(name="consts", bufs=1))
        xpool = ctx.enter_context(tc.tile_pool(name="xpool", bufs=1))
        qkpool = ctx.enter_context(tc.tile_pool(name="qkpool", bufs=1))
        vpool = ctx.enter_context(tc.tile_pool(name="vpool", bufs=1))
        silupool = ctx.enter_context(tc.tile_pool(name="silupool", bufs=1))
        biaspool = ctx.enter_context(tc.tile_pool(name="biaspool", bufs=2))
        ppool = ctx.enter_context(tc.tile_pool(name="ppool", bufs=2))
        smalls = ctx.enter_context(tc.tile_pool(name="smalls", bufs=4))
        tpool = ctx.enter_context(tc.tile_pool(name="tpool", bufs=2))
        psbig = ctx.enter_context(tc.tile_pool(name="psbig", bufs=2, space="PSUM"))
        pssm = ctx.enter_context(tc.tile_pool(name="pssm", bufs=4, space="PSUM"))

        # ---- constants ----
        wqk_sb = consts.tile([128, 4, 1024], bf)
        nc.sync.dma_start(out=wqk_sb, in_=wqk)
        wv_sb = consts.tile([128, 4, H * VD], bf)
        nc.sync.dma_start(out=wv_sb, in_=wv)
        wp_sb = consts.tile([128, 16, DIM], bf)
        nc.sync.dma_start(out=wp_sb, in_=wp)
        bqk_sb = consts.tile([128, 8], f32)
        nc.sync.dma_start(out=bqk_sb, in_=bqk)
        bp_sb = consts.tile([1, DIM], bf)
        nc.sync.dma_start(out=bp_sb, in_=bp)
        ones_sb = consts.tile([1, 128], bf)
        nc.sync.dma_start(out=ones_sb, in_=ones)
        ident_sb = consts.tile([128, 128], bf)
        nc.sync.dma_start(out=ident_sb, in_=ident)
        bv_sb = consts.tile([1, H * VD], bf)
        nc.sync.dma_start(out=bv_sb, in_=bv)

        for b in range(BL):
            # ---- load xT[b]: [512, 784] -> [128, cc, 784] ----
            xT_sb = xpool.tile([128, 4, N], bf)
            xin = bass.AP(
                tensor=xT.tensor,
                offset=xT.offset + b * DIM * N,
                ap=[[N, 128], [128 * N, 4], [1, N]],
            )
            nc.sync.dma_start(out=xT_sb, in_=xin)

            # ---- pass A: qkT[o, t] for all heads (o-chunks 0-3 = q, 4-7 = k) ----
            qk_sb = qkpool.tile([128, 8, N], bf)
            for oc in range(8):
                ps = psbig.tile([128, N], f32, tag="big")
                for (i0, isz) in ITILES:
                    for cc in range(4):
                        nc.tensor.matmul(
                            ps[:, i0:i0 + isz],
                            lhsT=wqk_sb[:, cc, oc * 128:(oc + 1) * 128],
                            rhs=xT_sb[:, cc, i0:i0 + isz],
                            start=(cc == 0),
                            stop=(cc == 3),
                        )
                nc.vector.tensor_scalar_add(
                    out=qk_sb[:, oc, :], in0=ps, scalar1=bqk_sb[:, oc:oc + 1],
                )

            # ---- pass B: v[t, h*257+d']; ones cols via memset; bias via rank-1 ----
            v_sb = vpool.tile([128, 7, OVW], bf)
            v_resh = v_sb.rearrange("p t (h d) -> p t h d", d=VDA)
            nc.vector.memset(v_resh[:, :, :, VD:VDA], 1.0)
            for tc_i, (t0, tsz) in enumerate(CHUNKS):
                for ovt in range(4):
                    o0 = ovt * 512
                    ps = pssm.tile([128, 512], f32, tag="small")
                    nc.tensor.matmul(
                        ps[:tsz, :],
                        lhsT=ones_sb[0:1, :tsz],
                        rhs=bv_sb[0:1, o0:o0 + 512],
                        start=True,
                        stop=False,
                    )
                    for cc in range(4):
                        nc.tensor.matmul(
                            ps[:tsz, :],
                            lhsT=xT_sb[:, cc, t0:t0 + tsz],
                            rhs=wv_sb[:, cc, o0:o0 + 512],
                            start=False,
                            stop=(cc == 3),
                        )
                    # strided evict into the two 257-stride head blocks
                    nc.scalar.activation(
                        out=v_resh[:tsz, tc_i, 2 * ovt:2 * ovt + 2, :VD],
                        in_=ps[:tsz, :].rearrange("p (h d) -> p h d", d=VD),
                        func=AF.Copy,
                    )

            silu_sb = silupool.tile([128, 7, H * VD], bf)

            # ---- head pairs: S^T (row-group concurrent) -> exp -> AV -> normalize ----
            for hp in range(4):
                qoc, koc = hp, 4 + hp

                bias_sb = biaspool.tile([128, 7, 2, N], bf, tag="bt")
                for k in range(2):
                    bin_ = bass.AP(
                        tensor=biast.tensor,
                        offset=biast.offset + (2 * hp + k) * NJP * N,
                        ap=[[N, 128], [128 * N, 7], [1, N]],
                    )
                    nc.sync.dma_start(out=bias_sb[:, :, k, :], in_=bin_)

                p_sb = ppool.tile([128, 7, 2, N], bf)
                for jc, (j0, jsz) in enumerate(CHUNKS):
                    pse = psbig.tile([128, N], f32, tag="big")
                    pso = psbig.tile([128, N], f32, tag="big")
                    for (i0, isz) in ITILES:
                        nc.tensor.matmul(
                            pse[:jsz, i0:i0 + isz],
                            lhsT=qk_sb[0:64, koc, j0:j0 + jsz],
                            rhs=qk_sb[0:64, qoc, i0:i0 + isz],
                            start=True, stop=True,
                        )
                        nc.tensor.matmul(
                            pso[:jsz, i0:i0 + isz],
                            lhsT=qk_sb[64:128, koc, j0:j0 + jsz],
                            rhs=qk_sb[64:128, qoc, i0:i0 + isz],
                            start=True, stop=True,
                        )
                    for k, ps in ((0, pse), (1, pso)):
                        # exp(S)*exp(bias) == exp(S+bias); biast holds exp(bias)
                        nc.scalar.activation(
                            out=p_sb[:jsz, jc, k, :], in_=ps[:jsz, :],
                            func=AF.Exp,
                        )
                        nc.vector.tensor_tensor(
                            out=p_sb[:jsz, jc, k, :], in0=p_sb[:jsz, jc, k, :],
                            in1=bias_sb[:jsz, jc, k, :], op=MULT,
                        )

                for ic, (i0, isz) in enumerate(CHUNKS):
                    for k in range(2):
                        h = 2 * hp + k
                        ps = pssm.tile([128, 512], f32, tag="small")
                        for jc, (j0, jsz) in enumerate(CHUNKS):
                            nc.tensor.matmul(
                                ps[:isz, :VDA],
                                lhsT=p_sb[:jsz, jc, k, i0:i0 + isz],
                                rhs=v_sb[:jsz, jc, h * VDA:(h + 1) * VDA],
                                start=(jc == 0),
                                stop=(jc == 6),
                            )
                        rs = smalls.tile([128, 1], f32)
                        nc.vector.reciprocal(out=rs[:isz], in_=ps[:isz, VD:VDA])
                        # normalized pre-silu values (silu applied in bulk later)
                        nc.vector.tensor_scalar_mul(
                            out=silu_sb[:isz, ic, h * VD:(h + 1) * VD],
                            in0=ps[:isz, :VD], scalar1=rs[:isz, 0:1],
                        )

            # ---- proj: silu, transpose silu chunks, accumulate over 16 v-chunks ----
            for tc_i, (t0, tsz) in enumerate(CHUNKS):
                nc.scalar.activation(
                    out=silu_sb[:tsz, tc_i, :], in_=silu_sb[:tsz, tc_i, :],
                    func=AF.Silu,
                )
                psf = pssm.tile([128, 512], f32, tag="small")
                nc.tensor.matmul(
                    psf[:tsz, :],
                    lhsT=ones_sb[0:1, :tsz],
                    rhs=bp_sb[0:1, :],
                    start=True,
                    stop=False,
                )
                for vp in range(8):
                    pst = psbig.tile([128, 2, 128], bf, tag="big")
                    for k in range(2):
                        vc = vp * 2 + k
                        nc.tensor.transpose(
                            pst[:, k, :tsz],
                            silu_sb[:tsz, tc_i, vc * 128:(vc + 1) * 128],
                            ident_sb[:tsz, :tsz],
                        )
                    st = tpool.tile([128, 2, 128], bf)
                    nc.scalar.copy(out=st[:, :, :tsz], in_=pst[:, :, :tsz])
                    for k in range(2):
                        vc = vp * 2 + k
                        nc.tensor.matmul(
                            psf[:tsz, :],
                            lhsT=st[:, k, :tsz],
                            rhs=wp_sb[:, vc, :],
                            start=False,
                            stop=(vc == 15),
                        )
                fo = biaspool.tile([128, DIM], f32, tag="bt")
                nc.scalar.activation(out=fo[:tsz], in_=psf[:tsz], func=AF.Copy)
                nc.sync.dma_start(out=out[b, t0:t0 + tsz, :], in_=fo[:tsz])

    nc.finalize()
    return nc


def _prep(inputs):
    bf16 = ml_dtypes.bfloat16
    f32 = np.float32
    inputs = {k: np.asarray(v) for k, v in inputs.items()}

    s_qkv = (inputs["qkv_gamma"] / np.sqrt(inputs["qkv_var"] + EPS)).astype(f32)
    b_qkv = (inputs["qkv_beta"] - inputs["qkv_mean"] * s_qkv).astype(f32)
    w_fold = (inputs["qkv_w"] * s_qkv[:, None]).astype(f32)

    rows = np.arange((2 * KD + VD) * H).reshape(H, 2 * KD + VD)
    q_rows = rows[:, :KD].ravel()
    k_rows = rows[:, KD:2 * KD].ravel()
    v_rows = rows[:, 2 * KD:].ravel()

    wq = w_fold[q_rows] * SCALE
    bq = b_qkv[q_rows] * SCALE
    wk = w_fold[k_rows]
    bk = b_qkv[k_rows]
    wvm = w_fold[v_rows]
    bvm = b_qkv[v_rows]

    # wqk: [c, o] with o = [q(512), k(512)] -> [128, cc, 1024]
    wqkT = np.concatenate([wq, wk], axis=0).T.astype(bf16)          # [512, 1024]
    wqk_t = np.ascontiguousarray(wqkT.reshape(4, 128, 1024).transpose(1, 0, 2))
    bqk_t = np.concatenate([bq, bk]).reshape(8, 128).T.astype(f32)  # [128, 8]
    bqk_t = np.ascontiguousarray(bqk_t)

    wv_t = np.ascontiguousarray(
        wvm.T.astype(bf16).reshape(4, 128, H * VD).transpose(1, 0, 2)
    )
    bv_t = np.ascontiguousarray(bvm.astype(bf16)[None, :])

    s_p = (inputs["proj_gamma"] / np.sqrt(inputs["proj_var"] + EPS)).astype(f32)
    b_p = (inputs["proj_beta"] - inputs["proj_mean"] * s_p).astype(f32)
    wp_fold = (inputs["proj_w"] * s_p[:, None]).astype(f32)          # [512, 2048]
    wp_t = np.ascontiguousarray(
        wp_fold.T.astype(bf16).reshape(16, 128, DIM).transpose(1, 0, 2)
    )
    bp_t = np.ascontiguousarray(b_p.astype(bf16)[None, :])

    bias_full = inputs["attention_biases"][:, inputs["bias_idxs"]].astype(f32)  # [H, N, N]
    biast = np.zeros((H, NJP, N), dtype=bf16)
    biast[:, :N, :] = np.exp(bias_full).astype(bf16)   # multiplicative form

    xT = inputs["x"].transpose(0, 2, 1).astype(bf16)                 # [B, 512, 784]

    shared = {
        "wqk": wqk_t, "wv": wv_t, "wp": wp_t, "bqk": bqk_t,
        "bv": bv_t, "bp": bp_t, "biast": biast,
        "ones": np.ones((1, 128), dtype=bf16),
        "ident": np.eye(128, dtype=np.float32).astype(bf16),
    }
    in_maps = []
    for c in range(NCORES):
        m = dict(shared)
        m["xT"] = np.ascontiguousarray(xT[c * BL:(c + 1) * BL])
        in_maps.append(m)
    return in_maps


def kernel(trace=False, **inputs):
    from concourse import bass_utils

    if "nc" not in _CACHE:
        _CACHE["nc"] = _build_nc()
    nc = _CACHE["nc"]

    in_maps = _prep(inputs)
    res = bass_utils.run_bass_kernel_spmd(
        nc, in_maps, core_ids=list(range(NCORES)), trace=trace,
    )
    out = np.concatenate([r["out"] for r in res.results], axis=0)
    if trace:
        return out.astype(np.float32), res
    return out.astype(np.float32)



# revision 8
# speedup vs baseline: 1.1010x; 1.1010x over previous
# Fused attention block (LeViT-style) for Trainium2, 8 NeuronCores, data-parallel over batch.
#
# reference computation (B=16, N=784, DIM=512, H=8, KD=64, VD=256):
#   qkv = BN(x @ qkv_w.T); split q,k,v per head
#   attn = softmax(q @ k.T * KD**-0.5 + attention_biases[:, bias_idxs])
#   out  = BN(silu(attn @ v reshaped) @ proj_w.T)
#
# Strategy:
#  - batch-parallel: 2 batches per core, weights/bias tables replicated, no collectives
#  - BN folded into weights on host; softmax scale folded into q weights
#  - all matmul operands bf16 (PSUM accumulation fp32), softmax pipeline fp32
#  - scores computed transposed (S^T[j,i]); bias table is symmetric so bias adds unchanged
#  - softmax denominator from an extra ones-column in v (col 256 of each head block)
#  - unstabilized softmax (scores empirically bounded ~|10|, exp is safe in fp32)
#  - heads processed in even/odd pairs at PE row bases 0/64: the K=64 score matmuls
#    of the two heads occupy disjoint row groups and execute concurrently
#  - bias adds via DVE with host-broadcast tiles (no K=1 PE matmuls)
#  - cross-phase software pipelining in PE issue order so the PE never waits on
#    ACT exp: S0 chunks interleaved into pass B, S(hp+1) chunks interleaved into
#    AV(hp) chains, proj chunks interleaved into AV(3)

import numpy as np
import ml_dtypes

B, N, DIM = 16, 784, 512
H, KD, VD = 8, 64, 256
RES = 28
EPS = 1e-5
SCALE = KD ** -0.5
NCORES = 8
BL = B // NCORES          # batches per core
VDA = VD + 1              # v head block with ones column
OVW = H * VDA             # 2056
NJP = 896                 # padded j extent (7 * 128)

# t/j chunking over N=784: six 128-chunks + one 16-chunk
CHUNKS = [(i * 128, min(128, N - i * 128)) for i in range((N + 127) // 128)]
ITILES = [(0, 512), (512, N - 512)]   # free-dim tiles for 784 (<=512 per PSUM bank)

_CACHE = {}


def _build_nc():
    from contextlib import ExitStack
    import concourse.bacc as bacc
    import concourse.tile as tile
    from concourse import mybir

    bf = mybir.dt.bfloat16
    f32 = mybir.dt.float32
    AF = mybir.ActivationFunctionType
    MULT = mybir.AluOpType.mult
    ADD = mybir.AluOpType.add

    nc = bacc.Bacc("TRN2", target_bir_lowering=False, debug=False)

    xT = nc.dram_tensor("xT", [BL, DIM, N], bf, kind="ExternalInput").ap()
    wqk = nc.dram_tensor("wqk", [128, 4, 1024], bf, kind="ExternalInput").ap()
    wv = nc.dram_tensor("wv", [128, 4, H * VD], bf, kind="ExternalInput").ap()
    wp = nc.dram_tensor("wp", [128, 16, DIM], bf, kind="ExternalInput").ap()
    bqk = nc.dram_tensor("bqk", [128, 8], f32, kind="ExternalInput").ap()
    bvb = nc.dram_tensor("bvb", [128, 4, 2, VD], bf, kind="ExternalInput").ap()
    bpb = nc.dram_tensor("bpb", [128, DIM], bf, kind="ExternalInput").ap()
    biast = nc.dram_tensor("biast", [H, NJP, N], bf, kind="ExternalInput").ap()
    ident = nc.dram_tensor("ident", [128, 128], bf, kind="ExternalInput").ap()
    out = nc.dram_tensor("out", [BL, N, DIM], bf, kind="ExternalOutput").ap()

    import concourse.bass as bass

    with ExitStack() as ctx:
        tc = ctx.enter_context(tile.TileContext(nc))
        consts = ctx.enter_context(tc.tile_pool(name="consts", bufs=1))
        xpool = ctx.enter_context(tc.tile_pool(name="xpool", bufs=1))
        qkpool = ctx.enter_context(tc.tile_pool(name="qkpool", bufs=1))
        vpool = ctx.enter_context(tc.tile_pool(name="vpool", bufs=1))
        silupool = ctx.enter_context(tc.tile_pool(name="silupool", bufs=1))
        biaspool = ctx.enter_context(tc.tile_pool(name="biaspool", bufs=2))
        ppool = ctx.enter_context(tc.tile_pool(name="ppool", bufs=2))
        smalls = ctx.enter_context(tc.tile_pool(name="smalls", bufs=4))
        tpool = ctx.enter_context(tc.tile_pool(name="tpool", bufs=2))
        fopool = ctx.enter_context(tc.tile_pool(name="fopool", bufs=1))
        psbig = ctx.enter_context(tc.tile_pool(name="psbig", bufs=3, space="PSUM"))
        pssm = ctx.enter_context(tc.tile_pool(name="pssm", bufs=2, space="PSUM"))

        # ---- constants ----
        wqk_sb = consts.tile([128, 4, 1024], bf)
        nc.sync.dma_start(out=wqk_sb, in_=wqk)
        wv_sb = consts.tile([128, 4, H * VD], bf)
        nc.sync.dma_start(out=wv_sb, in_=wv)
        wp_sb = consts.tile([128, 16, DIM], bf)
        nc.sync.dma_start(out=wp_sb, in_=wp)
        bqk_sb = consts.tile([128, 8], f32)
        nc.sync.dma_start(out=bqk_sb, in_=bqk)
        bvb_sb = consts.tile([128, 4, 2, VD], bf)
        nc.sync.dma_start(out=bvb_sb, in_=bvb)
        bpb_sb = consts.tile([128, DIM], bf)
        nc.sync.dma_start(out=bpb_sb, in_=bpb)
        ident_sb = consts.tile([128, 128], bf)
        nc.sync.dma_start(out=ident_sb, in_=ident)

        # per-batch state handles (filled as phases are issued)
        st_x = {}
        st_bias = {}
        st_p = {}

        def dma_x(b):
            xT_sb = xpool.tile([128, 4, N], bf)
            xin = bass.AP(
                tensor=xT.tensor,
                offset=xT.offset + b * DIM * N,
                ap=[[N, 128], [128 * N, 4], [1, N]],
            )
            nc.sync.dma_start(out=xT_sb, in_=xin)
            st_x[b] = xT_sb

        def dma_bias(hp):
            bias_sb = biaspool.tile([128, 7, 2, N], bf, tag="bt")
            for k in range(2):
                bin_ = bass.AP(
                    tensor=biast.tensor,
                    offset=biast.offset + (2 * hp + k) * NJP * N,
                    ap=[[N, 128], [128 * N, 7], [1, N]],
                )
                nc.sync.dma_start(out=bias_sb[:, :, k, :], in_=bin_)
            st_bias[hp] = bias_sb

        def emit_a(b, qk_sb):
            # qkT[o, t] for all heads (o-chunks 0-3 = q, 4-7 = k)
            xT_sb = st_x[b]
            for oc in range(8):
                ps = psbig.tile([128, N], f32, tag="big")
                for (i0, isz) in ITILES:
                    for cc in range(4):
                        nc.tensor.matmul(
                            ps[:, i0:i0 + isz],
                            lhsT=wqk_sb[:, cc, oc * 128:(oc + 1) * 128],
                            rhs=xT_sb[:, cc, i0:i0 + isz],
                            start=(cc == 0),
                            stop=(cc == 3),
                        )
                nc.vector.tensor_scalar_add(
                    out=qk_sb[:, oc, :], in0=ps, scalar1=bqk_sb[:, oc:oc + 1],
                )

        def emit_b_chain(b, v_resh, tc_i, tsz_pair):
            # v[t, h*257+d'] for one (t-chunk, ovt) pair; bias via DVE add
            xT_sb = st_x[b]
            (t0, tsz), ovt = tsz_pair[0], tsz_pair[1]
            o0 = ovt * 512
            ps = pssm.tile([128, 512], f32, tag="small")
            for cc in range(4):
                nc.tensor.matmul(
                    ps[:tsz, :],
                    lhsT=xT_sb[:, cc, t0:t0 + tsz],
                    rhs=wv_sb[:, cc, o0:o0 + 512],
                    start=(cc == 0),
                    stop=(cc == 3),
                )
            nc.vector.tensor_tensor(
                out=v_resh[:tsz, tc_i, 2 * ovt:2 * ovt + 2, :VD],
                in0=ps[:tsz, :].rearrange("p (h d) -> p h d", d=VD),
                in1=bvb_sb[:tsz, ovt],
                op=ADD,
            )

        def emit_s_chunk(qk_sb, hp, jc):
            # one j-chunk of transposed scores for head pair hp, + exp + bias mult
            qoc, koc = hp, 4 + hp
            j0, jsz = CHUNKS[jc]
            p_sb = st_p[hp]
            bias_sb = st_bias[hp]
            pse = psbig.tile([128, N], f32, tag="big")
            pso = psbig.tile([128, N], f32, tag="big")
            for (i0, isz) in ITILES:
                nc.tensor.matmul(
                    pse[:jsz, i0:i0 + isz],
                    lhsT=qk_sb[0:64, koc, j0:j0 + jsz],
                    rhs=qk_sb[0:64, qoc, i0:i0 + isz],
                    start=True, stop=True,
                )
                nc.tensor.matmul(
                    pso[:jsz, i0:i0 + isz],
                    lhsT=qk_sb[64:128, koc, j0:j0 + jsz],
                    rhs=qk_sb[64:128, qoc, i0:i0 + isz],
                    start=True, stop=True,
                )
            for k, ps in ((0, pse), (1, pso)):
                # exp(S)*exp(bias) == exp(S+bias); biast holds exp(bias)
                nc.scalar.activation(
                    out=p_sb[:jsz, jc, k, :], in_=ps[:jsz, :],
                    func=AF.Exp,
                )
                nc.vector.tensor_tensor(
                    out=p_sb[:jsz, jc, k, :], in0=p_sb[:jsz, jc, k, :],
                    in1=bias_sb[:jsz, jc, k, :], op=MULT,
                )

        def emit_av_chunk(v_sb, silu_sb, hp, ic):
            # attn @ v for one i-chunk, both heads of the pair; normalize via DVE
            p_sb = st_p[hp]
            i0, isz = CHUNKS[ic]
            for k in range(2):
                h = 2 * hp + k
                ps = pssm.tile([128, 512], f32, tag="small")
                for jc, (j0, jsz) in enumerate(CHUNKS):
                    nc.tensor.matmul(
                        ps[:isz, :VDA],
                        lhsT=p_sb[:jsz, jc, k, i0:i0 + isz],
                        rhs=v_sb[:jsz, jc, h * VDA:(h + 1) * VDA],
                        start=(jc == 0),
                        stop=(jc == 6),
                    )
                rs = smalls.tile([128, 1], f32)
                nc.vector.reciprocal(out=rs[:isz], in_=ps[:isz, VD:VDA])
                # normalized pre-silu values (silu applied per t-chunk in proj)
                nc.vector.tensor_scalar_mul(
                    out=silu_sb[:isz, ic, h * VD:(h + 1) * VD],
                    in0=ps[:isz, :VD], scalar1=rs[:isz, 0:1],
                )

        def emit_proj_chunk(b, silu_sb, tc_i):
            # silu, transpose silu chunks, accumulate over 16 v-chunks, BN, store
            t0, tsz = CHUNKS[tc_i]
            nc.scalar.activation(
                out=silu_sb[:tsz, tc_i, :], in_=silu_sb[:tsz, tc_i, :],
                func=AF.Silu,
            )
            psf = pssm.tile([128, 512], f32, tag="small")
            for vp in range(8):
                pst = psbig.tile([128, 2, 128], bf, tag="big")
                for k in range(2):
                    vc = vp * 2 + k
                    nc.tensor.transpose(
                        pst[:, k, :tsz],
                        silu_sb[:tsz, tc_i, vc * 128:(vc + 1) * 128],
                        ident_sb[:tsz, :tsz],
                    )
                st = tpool.tile([128, 2, 128], bf)
                nc.vector.tensor_scalar_add(
                    out=st[:, :, :tsz], in0=pst[:, :, :tsz], scalar1=0.0,
                )
                for k in range(2):
                    vc = vp * 2 + k
                    nc.tensor.matmul(
                        psf[:tsz, :],
                        lhsT=st[:, k, :tsz],
                        rhs=wp_sb[:, vc, :],
                        start=(vc == 0),
                        stop=(vc == 15),
                    )
            fo = fopool.tile([128, DIM], bf)
            nc.vector.tensor_tensor(
                out=fo[:tsz], in0=psf[:tsz], in1=bpb_sb[:tsz], op=ADD,
            )
            nc.sync.dma_start(out=out[b, t0:t0 + tsz, :], in_=fo[:tsz])

        # ---- main schedule ----
        dma_x(0)
        dma_bias(0)
        dma_bias(1)
        for b in range(BL):
            qk_sb = qkpool.tile([128, 8, N], bf)
            emit_a(b, qk_sb)

            v_sb = vpool.tile([128, 7, OVW], bf)
            v_resh = v_sb.rearrange("p t (h d) -> p t h d", d=VDA)
            nc.vector.memset(v_resh[:, :, :, VD:VDA], 1.0)
            silu_sb = silupool.tile([128, 7, H * VD], bf)
            for hp in range(4):
                st_p[hp] = ppool.tile([128, 7, 2, N], bf, name=f"p_hp{hp}", tag="p")

            # pass B with S(0) chunks interleaved (1 score chunk per 4 v-chains)
            chains = [((t0, tsz), ovt) for (t0, tsz) in CHUNKS for ovt in range(4)]
            for i, pair in enumerate(chains):
                tc_i = i // 4
                emit_b_chain(b, v_resh, tc_i, pair)
                if i % 4 == 3:
                    emit_s_chunk(qk_sb, 0, i // 4)

            if b + 1 < BL:
                dma_x(b + 1)

            # AV(hp) interleaved with S(hp+1) at chunk granularity
            for hp in range(3):
                if hp + 2 < 4:
                    dma_bias(hp + 2)
                for ic in range(7):
                    emit_av_chunk(v_sb, silu_sb, hp, ic)
                    emit_s_chunk(qk_sb, hp + 1, ic)

            # AV(3) interleaved with proj chunks
            if b + 1 < BL:
                dma_bias(0)
                dma_bias(1)
            for ic in range(7):
                emit_av_chunk(v_sb, silu_sb, 3, ic)
                emit_proj_chunk(b, silu_sb, ic)

    nc.finalize()
    return nc


def _prep(inputs):
    bf16 = ml_dtypes.bfloat16
    f32 = np.float32
    inputs = {k: np.asarray(v) for k, v in inputs.items()}

    s_qkv = (inputs["qkv_gamma"] / np.sqrt(inputs["qkv_var"] + EPS)).astype(f32)
    b_qkv = (inputs["qkv_beta"] - inputs["qkv_mean"] * s_qkv).astype(f32)
    w_fold = (inputs["qkv_w"] * s_qkv[:, None]).astype(f32)

    rows = np.arange((2 * KD + VD) * H).reshape(H, 2 * KD + VD)
    q_rows = rows[:, :KD].ravel()
    k_rows = rows[:, KD:2 * KD].ravel()
    v_rows = rows[:, 2 * KD:].ravel()

    wq = w_fold[q_rows] * SCALE
    bq = b_qkv[q_rows] * SCALE
    wk = w_fold[k_rows]
    bk = b_qkv[k_rows]
    wvm = w_fold[v_rows]
    bvm = b_qkv[v_rows]

    # wqk: [c, o] with o = [q(512), k(512)] -> [128, cc, 1024]
    wqkT = np.concatenate([wq, wk], axis=0).T.astype(bf16)          # [512, 1024]
    wqk_t = np.ascontiguousarray(wqkT.reshape(4, 128, 1024).transpose(1, 0, 2))
    bqk_t = np.concatenate([bq, bk]).reshape(8, 128).T.astype(f32)  # [128, 8]
    bqk_t = np.ascontiguousarray(bqk_t)

    wv_t = np.ascontiguousarray(
        wvm.T.astype(bf16).reshape(4, 128, H * VD).transpose(1, 0, 2)
    )
    # v bias broadcast across partitions: [128, 4, 2, 256]
    bvb_t = np.ascontiguousarray(
        np.broadcast_to(bvm.astype(bf16).reshape(4, 2, VD), (128, 4, 2, VD))
    )

    s_p = (inputs["proj_gamma"] / np.sqrt(inputs["proj_var"] + EPS)).astype(f32)
    b_p = (inputs["proj_beta"] - inputs["proj_mean"] * s_p).astype(f32)
    wp_fold = (inputs["proj_w"] * s_p[:, None]).astype(f32)          # [512, 2048]
    wp_t = np.ascontiguousarray(
        wp_fold.T.astype(bf16).reshape(16, 128, DIM).transpose(1, 0, 2)
    )
    bpb_t = np.ascontiguousarray(
        np.broadcast_to(b_p.astype(bf16)[None, :], (128, DIM))
    )

    bias_full = inputs["attention_biases"][:, inputs["bias_idxs"]].astype(f32)  # [H, N, N]
    biast = np.zeros((H, NJP, N), dtype=bf16)
    biast[:, :N, :] = np.exp(bias_full).astype(bf16)   # multiplicative form

    xT = inputs["x"].transpose(0, 2, 1).astype(bf16)                 # [B, 512, 784]

    shared = {
        "wqk": wqk_t, "wv": wv_t, "wp": wp_t, "bqk": bqk_t,
        "bvb": bvb_t, "bpb": bpb_t, "biast": biast,
        "ident": np.eye(128, dtype=np.float32).astype(bf16),
    }
    in_maps = []
    for c in range(NCORES):
        m = dict(shared)
        m["xT"] = np.ascontiguousarray(xT[c * BL:(c + 1) * BL])
        in_maps.append(m)
    return in_maps


def kernel(trace=False, **inputs):
    from concourse import bass_utils

    if "nc" not in _CACHE:
        _CACHE["nc"] = _build_nc()
    nc = _CACHE["nc"]

    in_maps = _prep(inputs)
    res = bass_utils.run_bass_kernel_spmd(
        nc, in_maps, core_ids=list(range(NCORES)), trace=trace,
    )
    out = np.concatenate([r["out"] for r in res.results], axis=0)
    if trace:
        return out.astype(np.float32), res
    return out.astype(np.float32)
